# revision 2
# baseline (speedup 1.0000x reference)
"""GPT2 attention (B=2,S=2048,D=1024,H=16,hd=64, no causal mask) on 8 trn2 cores.

Sharding: core c handles batch b=c//4 and head-group g=c%4 (4 heads = 2 pairs).
w_attn columns split per head group (Q scaled by 1/sqrt(hd) on host); w_proj
rows split per head group; host sums the 4 partial c_proj outputs per batch.

Dataflow (all bf16 on SBUF, fp32 PSUM accumulation); ACT exp is the
bottleneck engine (128 x [128,1024] exp blocks ~1.03us each), so the whole
kernel is one fused pipeline built to keep the exp stream gap-free:
  host passes hidT [D,S] bf16 (pre-transposed) so no on-device transpose.
  qT/kT [128,S] per head-pair = wqk_pair.T @ hidT; kT[0]+qT[0]-chunk0 are
    computed as dt-waves that complete right after the last input DMA.
  V computed directly seq-major: V[k,f] = hidT_chunk.T @ wv -> vaug
    ([128,65] tiles per (head,kt); col 64 = ones for the softmax denom).
  scores: per (pair,qc512,kt): two row-tiled concurrent MMs (K=64 heads at
    array rows 0-63/64-127) -> sp[128,1024] f32 PSUM -> one ACT exp ->
    eb bf16 -> attnV MMs accumulate op[65,512] (row 64 = denominator).
  blocks 0/1 run scores/exp under the remaining stage-A chains (eb tiles
    buffered in SBUF), draining their attnV against blocks 2/3; blocks 4-7
    run sequentially with c_proj chains for finished q-columns woven in.
  normalize: op evacuated to SBUF fast (recycles PSUM), 1/d = exp(-ln d)
    on ACT (combined ln+exp table pinned once -> no table thrash), GPSIMD
    partition_broadcast, DVE mult -> obar (feature-major, pair-packed).
  c_proj per 128-query tile: K=128 chains over 2 pairs; bf16 partial
    outputs, host sums the 4 head-group partials per batch in f32.
"""

import sys

import numpy as np

if "/opt/trn_rl_repo" not in sys.path:
    sys.path.insert(0, "/opt/trn_rl_repo")

S = 2048
D = 1024
P = 128
NH = 4  # heads per core
HD = 64
N_CORES = 8

_CACHE = {}


def _build_program():
    import concourse.mybir as mybir
    from concourse import bacc
    from concourse.tile import TileContext

    bf16 = mybir.dt.bfloat16
    f32 = mybir.dt.float32
    AF = mybir.ActivationFunctionType
    ALU = mybir.AluOpType

    nc = bacc.Bacc(None, target_bir_lowering=False, debug=False)
    hidT = nc.declare_dram_parameter("hidT", [D, S], bf16, isOutput=False)
    wqkv = nc.declare_dram_parameter("wqkv", [D, 3 * NH * HD], bf16, isOutput=False)
    wp = nc.declare_dram_parameter("wp", [NH * HD, D], bf16, isOutput=False)
    out = nc.declare_dram_parameter("out", [S, D], bf16, isOutput=True)

    with TileContext(nc) as tc:
        with tc.tile_pool(name="const", bufs=1) as constp:
            vaug = constp.tile([P, NH * 16 * 65], bf16)
            # ones column (col 64 of each 65-block) for the softmax denom
            vaug_on = vaug[:, :].rearrange("p (n c) -> p n c", c=65)[:, :, 64:65]
            nc.gpsimd.memset(vaug_on, 1.0)

            hid_sb = [constp.tile([P, S], bf16, name=f"hid{i}") for i in range(8)]
            w_sb = [constp.tile([P, 768], bf16, name=f"w{i}") for i in range(8)]
            wp_sb = [constp.tile([P, D], bf16, name=f"wp{i}") for i in range(2)]
            qT = [constp.tile([P, S], bf16, name=f"qT{i}") for i in range(2)]
            kT = [constp.tile([P, S], bf16, name=f"kT{i}") for i in range(2)]
            obar = [constp.tile([P, S], bf16, name=f"ob{i}") for i in range(2)]

            # 3-way DMA split: hidT alternates the two HWDGE rings
            # (sync+scalar), weights ride the otherwise-idle SWDGE ring
            for i in range(8):
                nc.gpsimd.dma_start(out=w_sb[i][:], in_=wqkv[i * P : (i + 1) * P, :])
                eng = nc.sync if i % 2 == 0 else nc.scalar
                eng.dma_start(out=hid_sb[i][:], in_=hidT[i * P : (i + 1) * P, :])
            for p in range(2):
                nc.gpsimd.dma_start(
                    out=wp_sb[p][:], in_=wp[p * P : (p + 1) * P, :]
                )

            # ------------- fused stage A + B + C pipeline ----------------
            # PSUM plan: phase 1-2 (stage-A merge) qk 2 + v 2 + sp 4 = 8
            # banks; phase 3+ (main pipeline) sp 4 + oppp 4 = 8 banks.
            # ACT is the bottleneck engine (128 exps ~1.03us each): blocks 0
            # and 1 run scores/exp under the stage-A chains with their eb
            # tiles buffered in SBUF, then drain attnV against blocks 2/3.
            # Denominator reciprocal = exp(-ln d) on ACT with the combined
            # ln+exp table pinned once (no table thrash); op tiles are
            # evacuated to SBUF before any slow normalize work so PSUM slots
            # recycle fast.
            blocks = [(qc, p) for qc in range(4) for p in range(2)]
            ops_of = {}
            vaug4 = vaug[:, :].rearrange("p (h x) -> p h x", h=NH)

            from concourse.hw_specs import get_activation_tables

            table_names = list(get_activation_tables(nc.m.arch).keys())
            pin = mybir.InstLoadActFuncSet(
                name=nc.get_next_instruction_name(),
                ins=[],
                outs=[],
                act_func_set_id=table_names.index("natural_log_exp_and_others"),
            )
            nc.scalar.add_instruction(pin)

            with tc.tile_pool(name="ebpool", bufs=34) as ebp, \
                 tc.tile_pool(name="recpool", bufs=3) as recp, \
                 tc.tile_pool(name="rbsb", bufs=2) as rbsbp, \
                 tc.tile_pool(name="otpool", bufs=2) as otp, \
                 tc.tile_pool(name="spsum", bufs=2, space="PSUM") as spsum:

                def scores_exp(bi, kt):
                    qc, p = blocks[bi]
                    q0 = qc * 512
                    sp = spsum.tile([P, 1024], f32, tag="sp", name="sp")
                    for u in range(2):
                        r0 = u * HD
                        nc.tensor.matmul(
                            sp[:, u * 512 : (u + 1) * 512],
                            lhsT=kT[p][r0 : r0 + HD, kt * P : (kt + 1) * P],
                            rhs=qT[p][r0 : r0 + HD, q0 : q0 + 512],
                            start=True,
                            stop=True,
                        )
                    eb = ebp.tile([P, 1024], bf16, tag="eb", name="eb")
                    with nc.allow_low_precision(reason="bf16 exp"):
                        nc.scalar.activation(eb[:], sp[:], AF.Exp)
                    return eb

                def attnv(bi, kt, eb, oppp):
                    qc, p = blocks[bi]
                    if kt == 0:
                        ops_of[bi] = [
                            oppp.tile([65, 512], f32, tag="op", name=f"op{u}")
                            for u in range(2)
                        ]
                    for u in range(2):
                        base = ((2 * p + u) * 16 + kt) * 65
                        nc.tensor.matmul(
                            ops_of[bi][u][:],
                            lhsT=vaug[:, base : base + 65],
                            rhs=eb[:, u * 512 : (u + 1) * 512],
                            start=(kt == 0),
                            stop=(kt == 15),
                        )

                def emit_kt(bi, kt, oppp):
                    attnv(bi, kt, scores_exp(bi, kt), oppp)

                def emit_norm(bi):
                    norm_finish(bi, evac(bi))

                def evac(bi):
                    # evacuate both op accumulators into ONE SBUF tile: frees
                    # the PSUM slots for the next block, and puts the two
                    # denominator rows side by side so the reciprocal runs as
                    # a single [1,1024] ln + exp pair (2 ACT ops, not 4)
                    ou2 = recp.tile([65, 1024], f32, tag="ou", name="ou")
                    for u in range(2):
                        nc.vector.tensor_copy(
                            ou2[:, u * 512 : (u + 1) * 512], ops_of[bi][u][:]
                        )
                    del ops_of[bi]
                    return ou2

                def norm_finish(bi, ou2):
                    qc, p = blocks[bi]
                    q0 = qc * 512
                    rec = recp.tile([1, 1024], f32, tag="rec", name="rec")
                    nc.vector.reciprocal(rec[:], ou2[HD : HD + 1, :])
                    rb2 = rbsbp.tile([HD, 1024], f32, tag="rbsb", name="rb")
                    nc.gpsimd.partition_broadcast(
                        rb2[:], rec[0:1, :], channels=HD
                    )
                    for u in range(2):
                        with nc.allow_low_precision(reason="bf16 obar"):
                            nc.vector.tensor_tensor(
                                out=obar[p][u * HD : (u + 1) * HD, q0 : q0 + 512],
                                in0=ou2[0:HD, u * 512 : (u + 1) * 512],
                                in1=rb2[:, u * 512 : (u + 1) * 512],
                                op=ALU.mult,
                            )

                def stage_c_chain(qc, j, oppp, act_copy=False):
                    qt = qc * 4 + j
                    ot = otp.tile([P, D], bf16, tag="ot", name="ot")
                    for ec in range(2):
                        pp = oppp.tile([P, 512], f32, tag="op", name="pp")
                        for p in range(2):
                            nc.tensor.matmul(
                                pp[:],
                                lhsT=obar[p][:, qt * P : (qt + 1) * P],
                                rhs=wp_sb[p][:, ec * 512 : (ec + 1) * 512],
                                start=(p == 0),
                                stop=(p == 1),
                            )
                        with nc.allow_low_precision(reason="bf16 out"):
                            if act_copy and ec == 1:
                                # tail chains: split copies across ACT + DVE
                                nc.scalar.copy(
                                    out=ot[:, ec * 512 : (ec + 1) * 512],
                                    in_=pp[:],
                                )
                            else:
                                nc.vector.tensor_copy(
                                    ot[:, ec * 512 : (ec + 1) * 512], pp[:]
                                )
                    for ec in range(2):
                        nc.sync.dma_start(
                            out=out[qt * P : (qt + 1) * P, ec * 512 : (ec + 1) * 512],
                            in_=ot[:, ec * 512 : (ec + 1) * 512],
                        )

                # phase 1+2: stage A chains with blocks 0/1 scores/exp woven
                eb0, eb1 = {}, {}
                with tc.tile_pool(name="qkpsum", bufs=4, space="PSUM") as qkp:

                    def qk_mms(ps, col, q4, dts):
                        for dt_ in dts:
                            nc.tensor.matmul(
                                ps[:],
                                lhsT=w_sb[dt_][:, col : col + P],
                                rhs=hid_sb[dt_][:, q4 * 512 : (q4 + 1) * 512],
                                start=(dt_ == 0),
                                stop=(dt_ == 7),
                            )

                    def qk_quanta(col, dst, q4):
                        # chain split in two ~0.85us quanta so scores/exp can
                        # slip between them in the in-order PE stream
                        cell = {}

                        def qa():
                            ps = qkp.tile([P, 512], f32, tag="qk", name="ps")
                            cell["ps"] = ps
                            qk_mms(ps, col, q4, range(4))

                        def qb():
                            ps = cell["ps"]
                            qk_mms(ps, col, q4, range(4, 8))
                            with nc.allow_low_precision(reason="bf16 qkT"):
                                nc.vector.tensor_copy(
                                    dst[:, q4 * 512 : (q4 + 1) * 512], ps[:]
                                )

                        return [qa, qb]

                    def qk_chain(col, dst, q4):
                        for q in qk_quanta(col, dst, q4):
                            q()

                    def v_mms(vps, kt, dts):
                        for dt_ in dts:
                            nc.tensor.matmul(
                                vps[:],
                                lhsT=hid_sb[dt_][:, kt * P : (kt + 1) * P],
                                rhs=w_sb[dt_][:, 512:768],
                                start=(dt_ == 0),
                                stop=(dt_ == 7),
                            )

                    def v_quanta(kt):
                        cell = {}

                        def qa():
                            vps = qkp.tile(
                                [P, NH * HD], f32, tag="qk", name="vps"
                            )
                            cell["vps"] = vps
                            v_mms(vps, kt, range(4))

                        def qb():
                            vps = cell["vps"]
                            v_mms(vps, kt, range(4, 8))
                            src = vps[:, :].rearrange("p (h c) -> p h c", h=NH)
                            dst = vaug4[:, :, kt * 65 : kt * 65 + HD]
                            with nc.allow_low_precision(reason="bf16 V"):
                                nc.vector.tensor_copy(dst, src)

                        return [qa, qb]

                    # kT[0] via dt-waves: 4 chains accumulate in 4 PSUM
                    # slots, each wave only needs one more hidT DMA tile, so
                    # the chains complete right after the last DMA lands
                    kps = [
                        qkp.tile([P, 512], f32, tag="qk", name=f"kp{q4}")
                        for q4 in range(4)
                    ]
                    # 5th wave chain (qT[0] chunk 0) borrows an sp-pool slot
                    qp0 = spsum.tile([P, 512], f32, tag="sp", name="qp0")
                    for dt_ in range(8):
                        last = dt_ == 7
                        # dt7 wave ordered so the two tiles scores(0,0)
                        # needs (qp0, kT chunk 0) finish + copy first
                        q4s = (0, 1, 2, 3) if not last else (0,)
                        nc.tensor.matmul(
                            qp0[:],
                            lhsT=w_sb[dt_][:, 0:P],
                            rhs=hid_sb[dt_][:, 0:512],
                            start=(dt_ == 0),
                            stop=last,
                        )
                        for q4 in q4s:
                            nc.tensor.matmul(
                                kps[q4][:],
                                lhsT=w_sb[dt_][:, 256 : 256 + P],
                                rhs=hid_sb[dt_][:, q4 * 512 : (q4 + 1) * 512],
                                start=(dt_ == 0),
                                stop=last,
                            )
                    with nc.allow_low_precision(reason="bf16 qkT"):
                        # parallel evacuation: DVE + idle ACT, so scores(0,0)
                        # starts one copy earlier
                        nc.vector.tensor_copy(qT[0][:, 0:512], qp0[:])
                        nc.scalar.copy(out=kT[0][:, 0:512], in_=kps[0][:])
                    for q4 in range(1, 4):
                        nc.tensor.matmul(
                            kps[q4][:],
                            lhsT=w_sb[7][:, 256 : 256 + P],
                            rhs=hid_sb[7][:, q4 * 512 : (q4 + 1) * 512],
                            start=False,
                            stop=True,
                        )
                        with nc.allow_low_precision(reason="bf16 qkT"):
                            nc.vector.tensor_copy(
                                kT[0][:, q4 * 512 : (q4 + 1) * 512], kps[q4][:]
                            )
                    # remaining stage-A chains woven under the exp stream;
                    # block 1 trails block 0 by 6 steps
                    weave = [
                        (0, qT[0], 1), (384, kT[1], 0), (P, qT[1], 0),
                        (0, qT[0], 2), (384, kT[1], 1), (0, qT[0], 3),
                        (384, kT[1], 2), (384, kT[1], 3), (P, qT[1], 1),
                    ]
                    quanta = []
                    for s in range(16):
                        quanta += v_quanta(s)
                        if 2 <= s < 2 + len(weave):
                            quanta += qk_quanta(*weave[s - 2])
                    quanta += qk_quanta(P, qT[1], 2)
                    quanta += qk_quanta(P, qT[1], 3)
                    qpop = 0
                    for s in range(22):
                        want = (s + 1) * len(quanta) // 22
                        while qpop < want:
                            quanta[qpop]()
                            qpop += 1
                        if s < 16:
                            eb0[s] = scores_exp(0, s)
                        if 6 <= s:
                            eb1[s - 6] = scores_exp(1, s - 6)
                    while qpop < len(quanta):
                        quanta[qpop]()
                        qpop += 1

                # phase 3: qk/v pools closed -> 4 banks for op/pp
                with tc.tile_pool(name="oppp", bufs=4, space="PSUM") as oppp:
                    pend = []

                    def run_block(drain, body, cqueue, post=None):
                        for kt in range(16):
                            if drain is not None:
                                bi, ebs = drain
                                attnv(bi, kt, ebs.pop(kt), oppp)
                            if body is not None:
                                emit_kt(body, kt, oppp)
                            if post and kt in (2, 6):
                                post.pop(0)()
                            if cqueue and pend and kt % 4 == 3:
                                stage_c_chain(*pend.pop(0), oppp)

                    run_block((0, eb0), 2, False)
                    o0, o2 = evac(0), evac(2)
                    run_block(
                        (1, eb1), 3, False,
                        post=[
                            lambda: norm_finish(0, o0),
                            lambda: norm_finish(2, o2),
                        ],
                    )
                    o1, o3 = evac(1), evac(3)
                    pend += [(0, j) for j in range(4)]
                    run_block(
                        None, 4, True,
                        post=[
                            lambda: norm_finish(1, o1),
                            lambda: norm_finish(3, o3),
                        ],
                    )
                    o4 = evac(4)
                    pend += [(1, j) for j in range(4)]
                    run_block(
                        None, 5, True, post=[lambda: norm_finish(4, o4)]
                    )
                    o5 = evac(5)
                    pend += [(2, j) for j in range(4)]
                    run_block(
                        None, 6, True, post=[lambda: norm_finish(5, o5)]
                    )
                    o6 = evac(6)
                    run_block(
                        None, 7, True, post=[lambda: norm_finish(6, o6)]
                    )
                    emit_norm(7)
                    pend += [(3, j) for j in range(4)]
                    while pend:
                        stage_c_chain(*pend.pop(0), oppp, act_copy=True)

    nc.compile()
    return nc


def _get_nc():
    if "nc" not in _CACHE:
        _CACHE["nc"] = _build_program()
    return _CACHE["nc"]


def _shard_inputs(hidden_states, w_attn, w_proj):
    import ml_dtypes

    bf16 = ml_dtypes.bfloat16
    scale = 1.0 / np.sqrt(np.float32(HD))
    in_maps = []
    for c in range(N_CORES):
        b, g = divmod(c, 4)
        cs = slice(g * NH * HD, (g + 1) * NH * HD)
        wq = w_attn[:, 0:D][:, cs] * scale
        wk = w_attn[:, D : 2 * D][:, cs]
        wv = w_attn[:, 2 * D : 3 * D][:, cs]
        in_maps.append(
            {
                "hidT": np.ascontiguousarray(
                    hidden_states[b].T.astype(bf16)
                ),
                "wqkv": np.ascontiguousarray(
                    np.concatenate([wq, wk, wv], axis=1).astype(bf16)
                ),
                "wp": np.ascontiguousarray(w_proj[cs, :].astype(bf16)),
            }
        )
    return in_maps


def run(hidden_states, w_attn, w_proj, trace=False):
    from concourse.bass_utils import run_bass_kernel_spmd

    nc = _get_nc()
    in_maps = _shard_inputs(hidden_states, w_attn, w_proj)
    res = run_bass_kernel_spmd(nc, in_maps, list(range(N_CORES)), trace=trace)
    parts = [res.results[c]["out"].astype(np.float32) for c in range(N_CORES)]
    out = np.stack(
        [
            parts[0] + parts[1] + parts[2] + parts[3],
            parts[4] + parts[5] + parts[6] + parts[7],
        ]
    ).astype(np.float32)
    return out, res


def kernel(hidden_states, w_attn, w_proj):
    out, _ = run(
        np.asarray(hidden_states), np.asarray(w_attn), np.asarray(w_proj)
    )
    return out



# revision 3
# speedup vs baseline: 1.1707x; 1.1707x over previous
"""GPT2 attention (B=2,S=2048,D=1024,H=16,hd=64, no causal mask) on 8 trn2 cores.

Sharding: core c handles batch b=c//4 and head-group g=c%4 (4 heads = 2 pairs).
w_attn columns split per head group (Q scaled by 1/sqrt(hd) on host); w_proj
rows split per head group; host sums the 4 partial c_proj outputs per batch.

Dataflow (all bf16 on SBUF, fp32 PSUM accumulation); ACT exp is the
bottleneck engine (128 x [128,1024] exp blocks ~1.03us each), so the whole
kernel is one fused pipeline built to keep the exp stream gap-free:
  host passes hidT [D,S] bf16 (pre-transposed) so no on-device transpose.
  qT/kT [128,S] per head-pair = wqk_pair.T @ hidT; kT[0]+qT[0]-chunk0 are
    computed as dt-waves that complete right after the last input DMA.
  V computed directly seq-major: V[k,f] = hidT_chunk.T @ wv -> vaug
    ([128,65] tiles per (head,kt); col 64 = ones for the softmax denom).
  scores: per (pair,qc512,kt): two row-tiled concurrent MMs (K=64 heads at
    array rows 0-63/64-127) -> sp[128,1024] f32 PSUM -> one ACT exp ->
    eb bf16 -> attnV MMs accumulate op[65,512] (row 64 = denominator).
  blocks 0/1 run scores/exp under the remaining stage-A chains (eb tiles
    buffered in SBUF), draining their attnV against blocks 2/3; blocks 4-7
    run sequentially with c_proj chains for finished q-columns woven in.
  normalize: op evacuated to SBUF fast (recycles PSUM), 1/d = exp(-ln d)
    on ACT (combined ln+exp table pinned once -> no table thrash), GPSIMD
    partition_broadcast, DVE mult -> obar (feature-major, pair-packed).
  c_proj per 128-query tile: K=128 chains over 2 pairs; bf16 partial
    outputs, host sums the 4 head-group partials per batch in f32.
"""

import sys

import numpy as np

if "/opt/trn_rl_repo" not in sys.path:
    sys.path.insert(0, "/opt/trn_rl_repo")

S = 2048
D = 1024
P = 128
NH = 4  # heads per core
HD = 64
N_CORES = 8

_CACHE = {}


def _build_program():
    import concourse.mybir as mybir
    from concourse import bacc
    from concourse.tile import TileContext

    bf16 = mybir.dt.bfloat16
    f32 = mybir.dt.float32
    AF = mybir.ActivationFunctionType
    ALU = mybir.AluOpType

    nc = bacc.Bacc(None, target_bir_lowering=False, debug=False)
    hidT = nc.declare_dram_parameter("hidT", [D, S], bf16, isOutput=False)
    wqkv = nc.declare_dram_parameter("wqkv", [D, 3 * NH * HD], bf16, isOutput=False)
    wp = nc.declare_dram_parameter("wp", [NH * HD, D], bf16, isOutput=False)
    out = nc.declare_dram_parameter("out", [S, D], bf16, isOutput=True)

    with TileContext(nc) as tc:
        with tc.tile_pool(name="const", bufs=1) as constp:
            vaug = constp.tile([P, NH * 16 * 65], bf16)
            # ones column (col 64 of each 65-block) for the softmax denom
            vaug_on = vaug[:, :].rearrange("p (n c) -> p n c", c=65)[:, :, 64:65]
            nc.gpsimd.memset(vaug_on, 1.0)

            hid_sb = [constp.tile([P, S], bf16, name=f"hid{i}") for i in range(8)]
            w_sb = [constp.tile([P, 768], bf16, name=f"w{i}") for i in range(8)]
            wp_sb = [constp.tile([P, D], bf16, name=f"wp{i}") for i in range(2)]
            qT = [constp.tile([P, S], bf16, name=f"qT{i}") for i in range(2)]
            kT = [constp.tile([P, S], bf16, name=f"kT{i}") for i in range(2)]
            obar = [constp.tile([P, S], bf16, name=f"ob{i}") for i in range(2)]

            # 3-way DMA split: hidT alternates the two HWDGE rings
            # (sync+scalar), weights ride the otherwise-idle SWDGE ring
            for i in range(8):
                nc.gpsimd.dma_start(out=w_sb[i][:], in_=wqkv[i * P : (i + 1) * P, :])
                eng = nc.sync if i % 2 == 0 else nc.scalar
                eng.dma_start(out=hid_sb[i][:], in_=hidT[i * P : (i + 1) * P, :])
            for p in range(2):
                nc.gpsimd.dma_start(
                    out=wp_sb[p][:], in_=wp[p * P : (p + 1) * P, :]
                )

            # ------------- fused stage A + B + C pipeline ----------------
            # PSUM plan: phase 1-2 (stage-A merge) qk 2 + v 2 + sp 4 = 8
            # banks; phase 3+ (main pipeline) sp 4 + oppp 4 = 8 banks.
            # ACT is the bottleneck engine (128 exps ~1.03us each): blocks 0
            # and 1 run scores/exp under the stage-A chains with their eb
            # tiles buffered in SBUF, then drain attnV against blocks 2/3.
            # Denominator reciprocal = exp(-ln d) on ACT with the combined
            # ln+exp table pinned once (no table thrash); op tiles are
            # evacuated to SBUF before any slow normalize work so PSUM slots
            # recycle fast.
            blocks = [(qc, p) for qc in range(4) for p in range(2)]
            ops_of = {}
            vaug4 = vaug[:, :].rearrange("p (h x) -> p h x", h=NH)

            from concourse.hw_specs import get_activation_tables

            table_names = list(get_activation_tables(nc.m.arch).keys())
            pin = mybir.InstLoadActFuncSet(
                name=nc.get_next_instruction_name(),
                ins=[],
                outs=[],
                act_func_set_id=table_names.index("natural_log_exp_and_others"),
            )
            nc.scalar.add_instruction(pin)

            with tc.tile_pool(name="ebpool", bufs=34) as ebp, \
                 tc.tile_pool(name="recpool", bufs=3) as recp, \
                 tc.tile_pool(name="rbsb", bufs=2) as rbsbp, \
                 tc.tile_pool(name="otpool", bufs=2) as otp, \
                 tc.tile_pool(name="spsum", bufs=2, space="PSUM") as spsum:

                def scores_exp(bi, kt):
                    qc, p = blocks[bi]
                    q0 = qc * 512
                    sp = spsum.tile([P, 1024], f32, tag="sp", name="sp")
                    for u in range(2):
                        r0 = u * HD
                        nc.tensor.matmul(
                            sp[:, u * 512 : (u + 1) * 512],
                            lhsT=kT[p][r0 : r0 + HD, kt * P : (kt + 1) * P],
                            rhs=qT[p][r0 : r0 + HD, q0 : q0 + 512],
                            start=True,
                            stop=True,
                        )
                    eb = ebp.tile([P, 1024], bf16, tag="eb", name="eb")
                    with nc.allow_low_precision(reason="bf16 exp"):
                        nc.scalar.activation(eb[:], sp[:], AF.Exp)
                    return eb

                def attnv(bi, kt, eb, oppp):
                    qc, p = blocks[bi]
                    if kt == 0:
                        ops_of[bi] = [
                            oppp.tile([65, 512], f32, tag="op", name=f"op{u}")
                            for u in range(2)
                        ]
                    for u in range(2):
                        base = ((2 * p + u) * 16 + kt) * 65
                        nc.tensor.matmul(
                            ops_of[bi][u][:],
                            lhsT=vaug[:, base : base + 65],
                            rhs=eb[:, u * 512 : (u + 1) * 512],
                            start=(kt == 0),
                            stop=(kt == 15),
                        )

                def emit_kt(bi, kt, oppp):
                    attnv(bi, kt, scores_exp(bi, kt), oppp)

                def emit_norm(bi):
                    norm_finish(bi, evac(bi))

                def evac(bi):
                    # evacuate both op accumulators into ONE SBUF tile: frees
                    # the PSUM slots for the next block, and puts the two
                    # denominator rows side by side so the reciprocal runs as
                    # a single [1,1024] ln + exp pair (2 ACT ops, not 4)
                    ou2 = recp.tile([65, 1024], f32, tag="ou", name="ou")
                    for u in range(2):
                        nc.vector.tensor_copy(
                            ou2[:, u * 512 : (u + 1) * 512], ops_of[bi][u][:]
                        )
                    del ops_of[bi]
                    return ou2

                def norm_finish(bi, ou2):
                    qc, p = blocks[bi]
                    q0 = qc * 512
                    ln_t = recp.tile([1, 1024], f32, tag="ln", name="ln")
                    nc.scalar.activation(ln_t[:], ou2[HD : HD + 1, :], AF.Ln)
                    rec = recp.tile([1, 1024], f32, tag="rec", name="rec")
                    nc.scalar.activation(
                        rec[:], ln_t[0:1, :], AF.Exp, scale=-1.0
                    )
                    rb2 = rbsbp.tile([HD, 1024], f32, tag="rbsb", name="rb")
                    nc.gpsimd.partition_broadcast(
                        rb2[:], rec[0:1, :], channels=HD
                    )
                    for u in range(2):
                        with nc.allow_low_precision(reason="bf16 obar"):
                            nc.vector.tensor_tensor(
                                out=obar[p][u * HD : (u + 1) * HD, q0 : q0 + 512],
                                in0=ou2[0:HD, u * 512 : (u + 1) * 512],
                                in1=rb2[:, u * 512 : (u + 1) * 512],
                                op=ALU.mult,
                            )

                def stage_c_chain(qc, j, oppp, act_copy=False):
                    qt = qc * 4 + j
                    ot = otp.tile([P, D], bf16, tag="ot", name="ot")
                    for ec in range(2):
                        pp = oppp.tile([P, 512], f32, tag="op", name="pp")
                        for p in range(2):
                            nc.tensor.matmul(
                                pp[:],
                                lhsT=obar[p][:, qt * P : (qt + 1) * P],
                                rhs=wp_sb[p][:, ec * 512 : (ec + 1) * 512],
                                start=(p == 0),
                                stop=(p == 1),
                            )
                        with nc.allow_low_precision(reason="bf16 out"):
                            if act_copy and ec == 1:
                                # tail chains: split copies across ACT + DVE
                                nc.scalar.copy(
                                    out=ot[:, ec * 512 : (ec + 1) * 512],
                                    in_=pp[:],
                                )
                            else:
                                nc.vector.tensor_copy(
                                    ot[:, ec * 512 : (ec + 1) * 512], pp[:]
                                )
                    for ec in range(2):
                        nc.sync.dma_start(
                            out=out[qt * P : (qt + 1) * P, ec * 512 : (ec + 1) * 512],
                            in_=ot[:, ec * 512 : (ec + 1) * 512],
                        )

                # phase 1+2: stage A chains with blocks 0/1 scores/exp woven
                eb0, eb1 = {}, {}
                with tc.tile_pool(name="qkpsum", bufs=4, space="PSUM") as qkp:

                    def qk_mms(ps, col, q4, dts):
                        for dt_ in dts:
                            nc.tensor.matmul(
                                ps[:],
                                lhsT=w_sb[dt_][:, col : col + P],
                                rhs=hid_sb[dt_][:, q4 * 512 : (q4 + 1) * 512],
                                start=(dt_ == 0),
                                stop=(dt_ == 7),
                            )

                    def qk_quanta(col, dst, q4):
                        # chain split in two ~0.85us quanta so scores/exp can
                        # slip between them in the in-order PE stream
                        cell = {}

                        def qa():
                            ps = qkp.tile([P, 512], f32, tag="qk", name="ps")
                            cell["ps"] = ps
                            qk_mms(ps, col, q4, range(4))

                        def qb():
                            ps = cell["ps"]
                            qk_mms(ps, col, q4, range(4, 8))
                            with nc.allow_low_precision(reason="bf16 qkT"):
                                nc.vector.tensor_copy(
                                    dst[:, q4 * 512 : (q4 + 1) * 512], ps[:]
                                )

                        return [qa, qb]

                    def qk_chain(col, dst, q4):
                        for q in qk_quanta(col, dst, q4):
                            q()

                    def v_mms(vps, kt, dts):
                        for dt_ in dts:
                            nc.tensor.matmul(
                                vps[:],
                                lhsT=hid_sb[dt_][:, kt * P : (kt + 1) * P],
                                rhs=w_sb[dt_][:, 512:768],
                                start=(dt_ == 0),
                                stop=(dt_ == 7),
                            )

                    def v_quanta(kt):
                        cell = {}

                        def qa():
                            vps = qkp.tile(
                                [P, NH * HD], f32, tag="qk", name="vps"
                            )
                            cell["vps"] = vps
                            v_mms(vps, kt, range(4))

                        def qb():
                            vps = cell["vps"]
                            v_mms(vps, kt, range(4, 8))
                            src = vps[:, :].rearrange("p (h c) -> p h c", h=NH)
                            dst = vaug4[:, :, kt * 65 : kt * 65 + HD]
                            with nc.allow_low_precision(reason="bf16 V"):
                                nc.vector.tensor_copy(dst, src)

                        return [qa, qb]

                    # kT[0] via dt-waves: 4 chains accumulate in 4 PSUM
                    # slots, each wave only needs one more hidT DMA tile, so
                    # the chains complete right after the last DMA lands
                    kps = [
                        qkp.tile([P, 512], f32, tag="qk", name=f"kp{q4}")
                        for q4 in range(4)
                    ]
                    # 5th wave chain (qT[0] chunk 0) borrows an sp-pool slot
                    qp0 = spsum.tile([P, 512], f32, tag="sp", name="qp0")
                    for dt_ in range(8):
                        last = dt_ == 7
                        # dt7 wave ordered so the two tiles scores(0,0)
                        # needs (qp0, kT chunk 0) finish + copy first
                        q4s = (0, 1, 2, 3) if not last else (0,)
                        nc.tensor.matmul(
                            qp0[:],
                            lhsT=w_sb[dt_][:, 0:P],
                            rhs=hid_sb[dt_][:, 0:512],
                            start=(dt_ == 0),
                            stop=last,
                        )
                        for q4 in q4s:
                            nc.tensor.matmul(
                                kps[q4][:],
                                lhsT=w_sb[dt_][:, 256 : 256 + P],
                                rhs=hid_sb[dt_][:, q4 * 512 : (q4 + 1) * 512],
                                start=(dt_ == 0),
                                stop=last,
                            )
                    with nc.allow_low_precision(reason="bf16 qkT"):
                        # parallel evacuation: DVE + idle ACT, so scores(0,0)
                        # starts one copy earlier
                        nc.vector.tensor_copy(qT[0][:, 0:512], qp0[:])
                        nc.scalar.copy(out=kT[0][:, 0:512], in_=kps[0][:])
                    for q4 in range(1, 4):
                        nc.tensor.matmul(
                            kps[q4][:],
                            lhsT=w_sb[7][:, 256 : 256 + P],
                            rhs=hid_sb[7][:, q4 * 512 : (q4 + 1) * 512],
                            start=False,
                            stop=True,
                        )
                        with nc.allow_low_precision(reason="bf16 qkT"):
                            nc.vector.tensor_copy(
                                kT[0][:, q4 * 512 : (q4 + 1) * 512], kps[q4][:]
                            )
                    # remaining stage-A chains woven under the exp stream;
                    # block 1 trails block 0 by 6 steps
                    weave = [
                        (0, qT[0], 1), (384, kT[1], 0), (P, qT[1], 0),
                        (0, qT[0], 2), (384, kT[1], 1), (0, qT[0], 3),
                        (384, kT[1], 2), (384, kT[1], 3), (P, qT[1], 1),
                    ]
                    quanta = []
                    for s in range(16):
                        quanta += v_quanta(s)
                        if 2 <= s < 2 + len(weave):
                            quanta += qk_quanta(*weave[s - 2])
                    quanta += qk_quanta(P, qT[1], 2)
                    quanta += qk_quanta(P, qT[1], 3)
                    qpop = 0
                    for s in range(22):
                        want = (s + 1) * len(quanta) // 22
                        while qpop < want:
                            quanta[qpop]()
                            qpop += 1
                        if s < 16:
                            eb0[s] = scores_exp(0, s)
                        if 6 <= s:
                            eb1[s - 6] = scores_exp(1, s - 6)
                    while qpop < len(quanta):
                        quanta[qpop]()
                        qpop += 1

                # phase 3: qk/v pools closed -> 4 banks for op/pp
                with tc.tile_pool(name="oppp", bufs=4, space="PSUM") as oppp:
                    pend = []

                    def run_block(drain, body, cqueue, post=None):
                        for kt in range(16):
                            if drain is not None:
                                bi, ebs = drain
                                attnv(bi, kt, ebs.pop(kt), oppp)
                            if body is not None:
                                emit_kt(body, kt, oppp)
                            if post and kt in (2, 6):
                                post.pop(0)()
                            if cqueue and pend and kt % 4 == 3:
                                stage_c_chain(*pend.pop(0), oppp)

                    run_block((0, eb0), 2, False)
                    o0, o2 = evac(0), evac(2)
                    run_block(
                        (1, eb1), 3, False,
                        post=[
                            lambda: norm_finish(0, o0),
                            lambda: norm_finish(2, o2),
                        ],
                    )
                    o1, o3 = evac(1), evac(3)
                    pend += [(0, j) for j in range(4)]
                    run_block(
                        None, 4, True,
                        post=[
                            lambda: norm_finish(1, o1),
                            lambda: norm_finish(3, o3),
                        ],
                    )
                    o4 = evac(4)
                    pend += [(1, j) for j in range(4)]
                    run_block(
                        None, 5, True, post=[lambda: norm_finish(4, o4)]
                    )
                    o5 = evac(5)
                    pend += [(2, j) for j in range(4)]
                    run_block(
                        None, 6, True, post=[lambda: norm_finish(5, o5)]
                    )
                    o6 = evac(6)
                    run_block(
                        None, 7, True, post=[lambda: norm_finish(6, o6)]
                    )
                    emit_norm(7)
                    pend += [(3, j) for j in range(4)]
                    while pend:
                        stage_c_chain(*pend.pop(0), oppp, act_copy=True)

    nc.compile()
    return nc


def _get_nc():
    if "nc" not in _CACHE:
        _CACHE["nc"] = _build_program()
    return _CACHE["nc"]


def _shard_inputs(hidden_states, w_attn, w_proj):
    import ml_dtypes

    bf16 = ml_dtypes.bfloat16
    scale = 1.0 / np.sqrt(np.float32(HD))
    in_maps = []
    for c in range(N_CORES):
        b, g = divmod(c, 4)
        cs = slice(g * NH * HD, (g + 1) * NH * HD)
        wq = w_attn[:, 0:D][:, cs] * scale
        wk = w_attn[:, D : 2 * D][:, cs]
        wv = w_attn[:, 2 * D : 3 * D][:, cs]
        in_maps.append(
            {
                "hidT": np.ascontiguousarray(
                    hidden_states[b].T.astype(bf16)
                ),
                "wqkv": np.ascontiguousarray(
                    np.concatenate([wq, wk, wv], axis=1).astype(bf16)
                ),
                "wp": np.ascontiguousarray(w_proj[cs, :].astype(bf16)),
            }
        )
    return in_maps


def run(hidden_states, w_attn, w_proj, trace=False):
    from concourse.bass_utils import run_bass_kernel_spmd

    nc = _get_nc()
    in_maps = _shard_inputs(hidden_states, w_attn, w_proj)
    res = run_bass_kernel_spmd(nc, in_maps, list(range(N_CORES)), trace=trace)
    parts = [res.results[c]["out"].astype(np.float32) for c in range(N_CORES)]
    out = np.stack(
        [
            parts[0] + parts[1] + parts[2] + parts[3],
            parts[4] + parts[5] + parts[6] + parts[7],
        ]
    ).astype(np.float32)
    return out, res


def kernel(hidden_states, w_attn, w_proj):
    out, _ = run(
        np.asarray(hidden_states), np.asarray(w_attn), np.asarray(w_proj)
    )
    return out



# revision 4
# speedup vs baseline: 1.1849x; 1.0122x over previous
"""GPT2 attention (B=2,S=2048,D=1024,H=16,hd=64, no causal mask) on 8 trn2 cores.

Sharding: core c handles batch b=c//4 and head-group g=c%4 (4 heads = 2 pairs).
w_attn columns split per head group (Q scaled by 1/sqrt(hd) on host); w_proj
rows split per head group; host sums the 4 partial c_proj outputs per batch.

Dataflow (all bf16 on SBUF, fp32 PSUM accumulation); ACT exp is the
bottleneck engine (128 x [128,1024] exp blocks ~1.03us each), so the whole
kernel is one fused pipeline built to keep the exp stream gap-free:
  host passes hidT [D,S] bf16 (pre-transposed) so no on-device transpose.
  qT/kT [128,S] per head-pair = wqk_pair.T @ hidT; kT[0]+qT[0]-chunk0 are
    computed as dt-waves that complete right after the last input DMA.
  V computed directly seq-major: V[k,f] = hidT_chunk.T @ wv -> vaug
    ([128,65] tiles per (head,kt); col 64 = ones for the softmax denom).
  scores: per (pair,qc512,kt): two row-tiled concurrent MMs (K=64 heads at
    array rows 0-63/64-127) -> sp[128,1024] f32 PSUM -> one ACT exp ->
    eb bf16 -> attnV MMs accumulate op[65,512] (row 64 = denominator).
  blocks 0/1 run scores/exp under the remaining stage-A chains (eb tiles
    buffered in SBUF), draining their attnV against blocks 2/3; blocks 4-7
    run sequentially with c_proj chains for finished q-columns woven in.
  normalize: op evacuated to SBUF fast (recycles PSUM), 1/d = exp(-ln d)
    on ACT (combined ln+exp table pinned once -> no table thrash), GPSIMD
    partition_broadcast, DVE mult -> obar (feature-major, pair-packed).
  c_proj per 128-query tile: K=128 chains over 2 pairs; bf16 partial
    outputs, host sums the 4 head-group partials per batch in f32.
"""

import sys

import numpy as np

if "/opt/trn_rl_repo" not in sys.path:
    sys.path.insert(0, "/opt/trn_rl_repo")

S = 2048
D = 1024
P = 128
NH = 4  # heads per core
HD = 64
N_CORES = 8

_CACHE = {}


def _build_program():
    import concourse.mybir as mybir
    from concourse import bacc
    from concourse.tile import TileContext

    bf16 = mybir.dt.bfloat16
    f32 = mybir.dt.float32
    AF = mybir.ActivationFunctionType
    ALU = mybir.AluOpType

    nc = bacc.Bacc(None, target_bir_lowering=False, debug=False)
    hidT = nc.declare_dram_parameter("hidT", [D, S], bf16, isOutput=False)
    wqkv = nc.declare_dram_parameter("wqkv", [D, 3 * NH * HD], bf16, isOutput=False)
    wp = nc.declare_dram_parameter("wp", [NH * HD, D], bf16, isOutput=False)
    out = nc.declare_dram_parameter("out", [S, D], bf16, isOutput=True)

    with TileContext(nc) as tc:
        with tc.tile_pool(name="const", bufs=1) as constp:
            vaug = constp.tile([P, NH * 16 * 65], bf16)
            # ones column (col 64 of each 65-block) for the softmax denom
            vaug_on = vaug[:, :].rearrange("p (n c) -> p n c", c=65)[:, :, 64:65]
            nc.gpsimd.memset(vaug_on, 1.0)

            hid_sb = [constp.tile([P, S], bf16, name=f"hid{i}") for i in range(8)]
            w_sb = [constp.tile([P, 768], bf16, name=f"w{i}") for i in range(8)]
            wp_sb = [constp.tile([P, D], bf16, name=f"wp{i}") for i in range(2)]
            qT = [constp.tile([P, S], bf16, name=f"qT{i}") for i in range(2)]
            kT = [constp.tile([P, S], bf16, name=f"kT{i}") for i in range(2)]
            obar = [constp.tile([P, S], bf16, name=f"ob{i}") for i in range(2)]

            # 3-way DMA split: hidT alternates the two HWDGE rings
            # (sync+scalar), weights ride the otherwise-idle SWDGE ring
            for i in range(8):
                nc.gpsimd.dma_start(out=w_sb[i][:], in_=wqkv[i * P : (i + 1) * P, :])
                eng = nc.sync if i % 2 == 0 else nc.scalar
                eng.dma_start(out=hid_sb[i][:], in_=hidT[i * P : (i + 1) * P, :])
            for p in range(2):
                nc.gpsimd.dma_start(
                    out=wp_sb[p][:], in_=wp[p * P : (p + 1) * P, :]
                )

            # ------------- fused stage A + B + C pipeline ----------------
            # PSUM plan: phase 1-2 (stage-A merge) qk 2 + v 2 + sp 4 = 8
            # banks; phase 3+ (main pipeline) sp 4 + oppp 4 = 8 banks.
            # ACT is the bottleneck engine (128 exps ~1.03us each): blocks 0
            # and 1 run scores/exp under the stage-A chains with their eb
            # tiles buffered in SBUF, then drain attnV against blocks 2/3.
            # Denominator reciprocal = exp(-ln d) on ACT with the combined
            # ln+exp table pinned once (no table thrash); op tiles are
            # evacuated to SBUF before any slow normalize work so PSUM slots
            # recycle fast.
            blocks = [(qc, p) for qc in range(4) for p in range(2)]
            ops_of = {}
            vaug4 = vaug[:, :].rearrange("p (h x) -> p h x", h=NH)

            from concourse.hw_specs import get_activation_tables

            table_names = list(get_activation_tables(nc.m.arch).keys())
            pin = mybir.InstLoadActFuncSet(
                name=nc.get_next_instruction_name(),
                ins=[],
                outs=[],
                act_func_set_id=table_names.index("natural_log_exp_and_others"),
            )
            nc.scalar.add_instruction(pin)

            with tc.tile_pool(name="ebpool", bufs=34) as ebp, \
                 tc.tile_pool(name="recpool", bufs=3) as recp, \
                 tc.tile_pool(name="rbsb", bufs=2) as rbsbp, \
                 tc.tile_pool(name="otpool", bufs=2) as otp, \
                 tc.tile_pool(name="spsum", bufs=2, space="PSUM") as spsum:

                def scores_exp(bi, kt):
                    qc, p = blocks[bi]
                    q0 = qc * 512
                    sp = spsum.tile([P, 1024], f32, tag="sp", name="sp")
                    for u in range(2):
                        r0 = u * HD
                        nc.tensor.matmul(
                            sp[:, u * 512 : (u + 1) * 512],
                            lhsT=kT[p][r0 : r0 + HD, kt * P : (kt + 1) * P],
                            rhs=qT[p][r0 : r0 + HD, q0 : q0 + 512],
                            start=True,
                            stop=True,
                        )
                    eb = ebp.tile([P, 1024], bf16, tag="eb", name="eb")
                    with nc.allow_low_precision(reason="bf16 exp"):
                        nc.scalar.activation(eb[:], sp[:], AF.Exp)
                    return eb

                def attnv(bi, kt, eb, oppp):
                    qc, p = blocks[bi]
                    if kt == 0:
                        ops_of[bi] = [
                            oppp.tile([65, 512], f32, tag="op", name=f"op{u}")
                            for u in range(2)
                        ]
                    for u in range(2):
                        base = ((2 * p + u) * 16 + kt) * 65
                        nc.tensor.matmul(
                            ops_of[bi][u][:],
                            lhsT=vaug[:, base : base + 65],
                            rhs=eb[:, u * 512 : (u + 1) * 512],
                            start=(kt == 0),
                            stop=(kt == 15),
                        )

                def emit_kt(bi, kt, oppp):
                    attnv(bi, kt, scores_exp(bi, kt), oppp)

                def emit_norm(bi):
                    norm_finish(bi, evac(bi))

                def evac(bi):
                    # evacuate both op accumulators into ONE SBUF tile: frees
                    # the PSUM slots for the next block, and puts the two
                    # denominator rows side by side so the reciprocal runs as
                    # a single [1,1024] ln + exp pair (2 ACT ops, not 4)
                    ou2 = recp.tile([65, 1024], f32, tag="ou", name="ou")
                    for u in range(2):
                        nc.vector.tensor_copy(
                            ou2[:, u * 512 : (u + 1) * 512], ops_of[bi][u][:]
                        )
                    del ops_of[bi]
                    return ou2

                def norm_finish(bi, ou2):
                    qc, p = blocks[bi]
                    q0 = qc * 512
                    ln_t = recp.tile([1, 1024], f32, tag="ln", name="ln")
                    nc.scalar.activation(ln_t[:], ou2[HD : HD + 1, :], AF.Ln)
                    rec = recp.tile([1, 1024], f32, tag="rec", name="rec")
                    nc.scalar.activation(
                        rec[:], ln_t[0:1, :], AF.Exp, scale=-1.0
                    )
                    rb2 = rbsbp.tile([HD, 1024], f32, tag="rbsb", name="rb")
                    nc.gpsimd.partition_broadcast(
                        rb2[:], rec[0:1, :], channels=HD
                    )
                    for u in range(2):
                        with nc.allow_low_precision(reason="bf16 obar"):
                            nc.vector.tensor_tensor(
                                out=obar[p][u * HD : (u + 1) * HD, q0 : q0 + 512],
                                in0=ou2[0:HD, u * 512 : (u + 1) * 512],
                                in1=rb2[:, u * 512 : (u + 1) * 512],
                                op=ALU.mult,
                            )

                def stage_c_half(qc, j, ec, ot, oppp, act_copy=False):
                    # one 512-col segment of a c_proj chain; emitting the two
                    # segments 2 kt-steps apart gives each pp->ot copy time
                    # to clear the DVE queue before its PSUM slot is reused
                    qt = qc * 4 + j
                    pp = oppp.tile([P, 512], f32, tag="op", name="pp")
                    for p in range(2):
                        nc.tensor.matmul(
                            pp[:],
                            lhsT=obar[p][:, qt * P : (qt + 1) * P],
                            rhs=wp_sb[p][:, ec * 512 : (ec + 1) * 512],
                            start=(p == 0),
                            stop=(p == 1),
                        )
                    with nc.allow_low_precision(reason="bf16 out"):
                        if act_copy and ec == 1:
                            nc.scalar.copy(
                                out=ot[:, ec * 512 : (ec + 1) * 512], in_=pp[:]
                            )
                        else:
                            nc.vector.tensor_copy(
                                ot[:, ec * 512 : (ec + 1) * 512], pp[:]
                            )
                    if ec == 1:
                        for e2 in range(2):
                            nc.sync.dma_start(
                                out=out[
                                    qt * P : (qt + 1) * P, e2 * 512 : (e2 + 1) * 512
                                ],
                                in_=ot[:, e2 * 512 : (e2 + 1) * 512],
                            )

                def stage_c_chain(qc, j, oppp, act_copy=False):
                    ot = otp.tile([P, D], bf16, tag="ot", name="ot")
                    for ec in range(2):
                        stage_c_half(qc, j, ec, ot, oppp, act_copy=act_copy)

                # phase 1+2: stage A chains with blocks 0/1 scores/exp woven
                eb0, eb1 = {}, {}
                with tc.tile_pool(name="qkpsum", bufs=4, space="PSUM") as qkp:

                    def qk_mms(ps, col, q4, dts):
                        for dt_ in dts:
                            nc.tensor.matmul(
                                ps[:],
                                lhsT=w_sb[dt_][:, col : col + P],
                                rhs=hid_sb[dt_][:, q4 * 512 : (q4 + 1) * 512],
                                start=(dt_ == 0),
                                stop=(dt_ == 7),
                            )

                    def qk_quanta(col, dst, q4):
                        # chain split in two ~0.85us quanta so scores/exp can
                        # slip between them in the in-order PE stream
                        cell = {}

                        def qa():
                            ps = qkp.tile([P, 512], f32, tag="qk", name="ps")
                            cell["ps"] = ps
                            qk_mms(ps, col, q4, range(4))

                        def qb():
                            ps = cell["ps"]
                            qk_mms(ps, col, q4, range(4, 8))
                            with nc.allow_low_precision(reason="bf16 qkT"):
                                nc.vector.tensor_copy(
                                    dst[:, q4 * 512 : (q4 + 1) * 512], ps[:]
                                )

                        return [qa, qb]

                    def qk_chain(col, dst, q4):
                        for q in qk_quanta(col, dst, q4):
                            q()

                    def v_mms(vps, kt, dts):
                        for dt_ in dts:
                            nc.tensor.matmul(
                                vps[:],
                                lhsT=hid_sb[dt_][:, kt * P : (kt + 1) * P],
                                rhs=w_sb[dt_][:, 512:768],
                                start=(dt_ == 0),
                                stop=(dt_ == 7),
                            )

                    def v_quanta(kt):
                        cell = {}

                        def qa():
                            vps = qkp.tile(
                                [P, NH * HD], f32, tag="qk", name="vps"
                            )
                            cell["vps"] = vps
                            v_mms(vps, kt, range(4))

                        def qb():
                            vps = cell["vps"]
                            v_mms(vps, kt, range(4, 8))
                            src = vps[:, :].rearrange("p (h c) -> p h c", h=NH)
                            dst = vaug4[:, :, kt * 65 : kt * 65 + HD]
                            with nc.allow_low_precision(reason="bf16 V"):
                                nc.vector.tensor_copy(dst, src)

                        return [qa, qb]

                    # kT[0] via dt-waves: 4 chains accumulate in 4 PSUM
                    # slots, each wave only needs one more hidT DMA tile, so
                    # the chains complete right after the last DMA lands
                    kps = [
                        qkp.tile([P, 512], f32, tag="qk", name=f"kp{q4}")
                        for q4 in range(4)
                    ]
                    # 5th wave chain (qT[0] chunk 0) borrows an sp-pool slot
                    qp0 = spsum.tile([P, 512], f32, tag="sp", name="qp0")
                    for dt_ in range(8):
                        last = dt_ == 7
                        # dt7 wave ordered so the two tiles scores(0,0)
                        # needs (qp0, kT chunk 0) finish + copy first
                        q4s = (0, 1, 2, 3) if not last else (0,)
                        nc.tensor.matmul(
                            qp0[:],
                            lhsT=w_sb[dt_][:, 0:P],
                            rhs=hid_sb[dt_][:, 0:512],
                            start=(dt_ == 0),
                            stop=last,
                        )
                        for q4 in q4s:
                            nc.tensor.matmul(
                                kps[q4][:],
                                lhsT=w_sb[dt_][:, 256 : 256 + P],
                                rhs=hid_sb[dt_][:, q4 * 512 : (q4 + 1) * 512],
                                start=(dt_ == 0),
                                stop=last,
                            )
                    with nc.allow_low_precision(reason="bf16 qkT"):
                        # parallel evacuation: DVE + idle ACT, so scores(0,0)
                        # starts one copy earlier
                        nc.vector.tensor_copy(qT[0][:, 0:512], qp0[:])
                        nc.scalar.copy(out=kT[0][:, 0:512], in_=kps[0][:])
                    for q4 in range(1, 4):
                        nc.tensor.matmul(
                            kps[q4][:],
                            lhsT=w_sb[7][:, 256 : 256 + P],
                            rhs=hid_sb[7][:, q4 * 512 : (q4 + 1) * 512],
                            start=False,
                            stop=True,
                        )
                        with nc.allow_low_precision(reason="bf16 qkT"):
                            nc.vector.tensor_copy(
                                kT[0][:, q4 * 512 : (q4 + 1) * 512], kps[q4][:]
                            )
                    # remaining stage-A chains woven under the exp stream;
                    # block 1 trails block 0 by 6 steps
                    weave = [
                        (0, qT[0], 1), (384, kT[1], 0), (P, qT[1], 0),
                        (0, qT[0], 2), (384, kT[1], 1), (0, qT[0], 3),
                        (384, kT[1], 2), (384, kT[1], 3), (P, qT[1], 1),
                    ]
                    quanta = []
                    for s in range(16):
                        quanta += v_quanta(s)
                        if 2 <= s < 2 + len(weave):
                            quanta += qk_quanta(*weave[s - 2])
                    quanta += qk_quanta(P, qT[1], 2)
                    quanta += qk_quanta(P, qT[1], 3)
                    qpop = 0
                    for s in range(22):
                        want = (s + 1) * len(quanta) // 22
                        while qpop < want:
                            quanta[qpop]()
                            qpop += 1
                        if s < 16:
                            eb0[s] = scores_exp(0, s)
                        if 6 <= s:
                            eb1[s - 6] = scores_exp(1, s - 6)
                    while qpop < len(quanta):
                        quanta[qpop]()
                        qpop += 1

                # phase 3: qk/v pools closed -> 4 banks for op/pp
                with tc.tile_pool(name="oppp", bufs=4, space="PSUM") as oppp:
                    pend = []
                    half_q = []

                    def run_block(drain, body, cqueue, post=None):
                        for kt in range(16):
                            if drain is not None:
                                bi, ebs = drain
                                attnv(bi, kt, ebs.pop(kt), oppp)
                            if body is not None:
                                emit_kt(body, kt, oppp)
                            if post and kt in (2, 6):
                                post.pop(0)()
                            if cqueue and kt % 2 == 1:
                                if half_q:
                                    half_q.pop(0)()
                                elif pend and kt >= 3:
                                    qc_, j_ = pend.pop(0)
                                    ot_ = otp.tile(
                                        [P, D], bf16, tag="ot", name="ot"
                                    )
                                    stage_c_half(qc_, j_, 0, ot_, oppp)
                                    half_q.append(
                                        lambda qc_=qc_, j_=j_, ot_=ot_:
                                        stage_c_half(qc_, j_, 1, ot_, oppp)
                                    )

                    run_block((0, eb0), 2, False)
                    o0, o2 = evac(0), evac(2)
                    run_block(
                        (1, eb1), 3, False,
                        post=[
                            lambda: norm_finish(0, o0),
                            lambda: norm_finish(2, o2),
                        ],
                    )
                    o1, o3 = evac(1), evac(3)
                    pend += [(0, j) for j in range(4)]
                    run_block(
                        None, 4, True,
                        post=[
                            lambda: norm_finish(1, o1),
                            lambda: norm_finish(3, o3),
                        ],
                    )
                    o4 = evac(4)
                    pend += [(1, j) for j in range(4)]
                    run_block(
                        None, 5, True, post=[lambda: norm_finish(4, o4)]
                    )
                    o5 = evac(5)
                    pend += [(2, j) for j in range(4)]
                    run_block(
                        None, 6, True, post=[lambda: norm_finish(5, o5)]
                    )
                    o6 = evac(6)
                    run_block(
                        None, 7, True, post=[lambda: norm_finish(6, o6)]
                    )
                    emit_norm(7)
                    pend += [(3, j) for j in range(4)]
                    while pend:
                        stage_c_chain(*pend.pop(0), oppp, act_copy=True)

    nc.compile()
    return nc


def _get_nc():
    if "nc" not in _CACHE:
        _CACHE["nc"] = _build_program()
    return _CACHE["nc"]


def _shard_inputs(hidden_states, w_attn, w_proj):
    import ml_dtypes

    bf16 = ml_dtypes.bfloat16
    scale = 1.0 / np.sqrt(np.float32(HD))
    in_maps = []
    for c in range(N_CORES):
        b, g = divmod(c, 4)
        cs = slice(g * NH * HD, (g + 1) * NH * HD)
        wq = w_attn[:, 0:D][:, cs] * scale
        wk = w_attn[:, D : 2 * D][:, cs]
        wv = w_attn[:, 2 * D : 3 * D][:, cs]
        in_maps.append(
            {
                "hidT": np.ascontiguousarray(
                    hidden_states[b].T.astype(bf16)
                ),
                "wqkv": np.ascontiguousarray(
                    np.concatenate([wq, wk, wv], axis=1).astype(bf16)
                ),
                "wp": np.ascontiguousarray(w_proj[cs, :].astype(bf16)),
            }
        )
    return in_maps


def run(hidden_states, w_attn, w_proj, trace=False):
    from concourse.bass_utils import run_bass_kernel_spmd

    nc = _get_nc()
    in_maps = _shard_inputs(hidden_states, w_attn, w_proj)
    res = run_bass_kernel_spmd(nc, in_maps, list(range(N_CORES)), trace=trace)
    parts = [res.results[c]["out"].astype(np.float32) for c in range(N_CORES)]
    out = np.stack(
        [
            parts[0] + parts[1] + parts[2] + parts[3],
            parts[4] + parts[5] + parts[6] + parts[7],
        ]
    ).astype(np.float32)
    return out, res


def kernel(hidden_states, w_attn, w_proj):
    out, _ = run(
        np.asarray(hidden_states), np.asarray(w_attn), np.asarray(w_proj)
    )
    return out

